# revision 6
# baseline (speedup 1.0000x reference)
"""ConvLSTM cell on 8 Trainium2 NeuronCores.

Problem: N=256 batch, DIN=256 in-channels, DH=512 hidden channels, 8x8 spatial,
two 3x3 convs (x->4*DH and h->4*DH, reflect-padded), LSTM gates with peephole.

Strategy:
  - Data-parallel over batch: 32 batch elements per core; weights replicated.
  - Host-side (numpy, untimed): reflect-pad x/h0, transpose all activations to
    [channel, batch*spatial] layout, cast conv operands to bf16, pre-transpose
    conv weights into per-tap [cin, cout] matmul tiles.
  - On-chip: conv as implicit GEMM — for each 3x3 tap, matmul over channel
    chunks, accumulating the 54 (9 taps * (2+4) cin chunks) partial products
    per output tile in PSUM.  Pointwise LSTM math on Vector/Scalar engines.
  - Gather: concatenate per-core outputs, transpose back to [n, ch, 8, 8].
"""
import numpy as np
import ml_dtypes

import concourse.bass as bass
import concourse.mybir as mybir
import concourse.tile as tile
from concourse import bacc
from concourse.bass_utils import run_bass_kernel_spmd

bf16 = ml_dtypes.bfloat16
F32 = mybir.dt.float32
BF = mybir.dt.bfloat16

N_CORES = 8
N, DIN, DH, W = 256, 256, 512, 8
NB = N // N_CORES            # 32 batch per core
PAD = 100                    # 10*10 reflect-padded image, flattened
POS = W * W                  # 64
FREE = NB * POS              # 2048 free positions per core
CI_X = DIN // 128            # 2 cin chunks for x-conv
CI_H = DH // 128             # 4 cin chunks for h-conv
NJ = DH // 128               # 4 hidden-channel chunks
NF = 4                       # free chunks of 512
FN = NB // NF                # 8 batch elems per free chunk
FW = FREE // NF              # 512

SIG = mybir.ActivationFunctionType.Sigmoid
TANH = mybir.ActivationFunctionType.Tanh
ADD = mybir.AluOpType.add


def _build_nc():
    nc = bacc.Bacc("TRN2", target_bir_lowering=False, debug=False,
                   num_devices=N_CORES)

    xp_d = nc.dram_tensor("xp", [DIN, NB * PAD], BF, kind="ExternalInput")
    hp_d = nc.dram_tensor("hp", [DH, NB * PAD], BF, kind="ExternalInput")
    c0_d = nc.dram_tensor("c0", [DH, FREE], F32, kind="ExternalInput")
    wx_d = nc.dram_tensor("wx", [4 * NJ, CI_X, 128, 9 * 128], BF,
                          kind="ExternalInput")
    wh_d = nc.dram_tensor("wh", [4 * NJ, CI_H, 128, 9 * 128], BF,
                          kind="ExternalInput")
    wci_d = nc.dram_tensor("wci", [DH, FW], BF, kind="ExternalInput")
    wcf_d = nc.dram_tensor("wcf", [DH, FW], BF, kind="ExternalInput")
    wco_d = nc.dram_tensor("wco", [DH, FW], BF, kind="ExternalInput")
    b_d = nc.dram_tensor("b", [4 * DH, 1], F32, kind="ExternalInput")
    o_d = nc.dram_tensor("o", [DH, FREE], F32, kind="ExternalOutput")
    ht_d = nc.dram_tensor("ht", [DH, FREE], F32, kind="ExternalOutput")
    ct_d = nc.dram_tensor("ct", [DH, FREE], F32, kind="ExternalOutput")

    with tile.TileContext(nc) as tc:
        _body(nc, tc, xp_d, hp_d, c0_d, wx_d, wh_d, wci_d, wcf_d, wco_d, b_d,
              o_d, ht_d, ct_d)
    nc.compile()
    return nc


def _body(nc, tc, xp_d, hp_d, c0_d, wx_d, wh_d, wci_d, wcf_d, wco_d, b_d,
          o_d, ht_d, ct_d):
    with (
        tc.tile_pool(name="res", bufs=1) as res,       # resident activations
        tc.tile_pool(name="wp", bufs=2) as wp,         # streamed weights
        tc.tile_pool(name="gates", bufs=2) as gp,      # per-j gate planes
        tc.tile_pool(name="tmp", bufs=2) as tp,        # per-f temporaries
        tc.tile_pool(name="outs", bufs=4) as op,       # output staging
        tc.tile_pool(name="ps", bufs=4, space="PSUM") as ps,
    ):
        # ---- resident loads -------------------------------------------------
        xp_sb = []
        for ci in range(CI_X):
            t = res.tile([128, NB * PAD], BF, tag=f"xp{ci}")
            nc.sync.dma_start(out=t, in_=xp_d[ci * 128:(ci + 1) * 128, :])
            xp_sb.append(t.rearrange("p (n r c) -> p n r c", n=NB, r=10, c=10))
        hp_sb = []
        for ci in range(CI_H):
            t = res.tile([128, NB * PAD], BF, tag=f"hp{ci}")
            nc.sync.dma_start(out=t, in_=hp_d[ci * 128:(ci + 1) * 128, :])
            hp_sb.append(t.rearrange("p (n r c) -> p n r c", n=NB, r=10, c=10))
        c0_sb = []
        for j in range(NJ):
            t = res.tile([128, FREE], F32, tag=f"c0{j}")
            nc.sync.dma_start(out=t, in_=c0_d[j * 128:(j + 1) * 128, :])
            c0_sb.append(t)
        wc_sb = {}
        for name, d in (("i", wci_d), ("f", wcf_d), ("o", wco_d)):
            for j in range(NJ):
                t = res.tile([128, FW], BF, tag=f"wc{name}{j}")
                nc.sync.dma_start(out=t, in_=d[j * 128:(j + 1) * 128, :])
                wc_sb[name, j] = t
        b_sb = []
        for cc in range(4 * NJ):
            t = res.tile([128, 1], F32, tag=f"b{cc}")
            nc.sync.dma_start(out=t, in_=b_d[cc * 128:(cc + 1) * 128, :])
            b_sb.append(t)

        # ---- main loop ------------------------------------------------------
        for j in range(NJ):
            # per-j gate planes (full 2048 free) in bf16
            i_pl = gp.tile([128, FREE], BF, tag="i_pl")
            f_pl = gp.tile([128, FREE], BF, tag="f_pl")
            g_pl = gp.tile([128, FREE], BF, tag="g_pl")

            for gate in range(4):          # 0:i 1:f 2:g 3:o
                cc = gate * NJ + j
                wx_sb = []
                for ci in range(CI_X):
                    t = wp.tile([128, 9 * 128], BF, tag=f"wx{ci}")
                    nc.sync.dma_start(out=t, in_=wx_d[cc, ci, :, :])
                    wx_sb.append(t)
                wh_sb = []
                for ci in range(CI_H):
                    t = wp.tile([128, 9 * 128], BF, tag=f"wh{ci}")
                    nc.sync.dma_start(out=t, in_=wh_d[cc, ci, :, :])
                    wh_sb.append(t)

                for f in range(NF):
                    p = ps.tile([128, FW], F32, tag="p")
                    k = 0
                    nk = 9 * (CI_X + CI_H)
                    for ci in range(CI_X):
                        for t9 in range(9):
                            ky, kx = t9 // 3, t9 % 3
                            rhs = xp_sb[ci][:, f * FN:(f + 1) * FN,
                                            ky:ky + 8, kx:kx + 8]
                            nc.tensor.matmul(
                                p[:, :], wx_sb[ci][:, t9 * 128:(t9 + 1) * 128],
                                rhs, start=(k == 0), stop=(k == nk - 1))
                            k += 1
                    for ci in range(CI_H):
                        for t9 in range(9):
                            ky, kx = t9 // 3, t9 % 3
                            rhs = hp_sb[ci][:, f * FN:(f + 1) * FN,
                                            ky:ky + 8, kx:kx + 8]
                            nc.tensor.matmul(
                                p[:, :], wh_sb[ci][:, t9 * 128:(t9 + 1) * 128],
                                rhs, start=(k == 0), stop=(k == nk - 1))
                            k += 1

                    fs = slice(f * FW, (f + 1) * FW)
                    js = slice(j * 128, (j + 1) * 128)
                    if gate == 0 or gate == 1:      # i / f: sigmoid(p + b + c0*Wc)
                        nm = "i" if gate == 0 else "f"
                        peep = tp.tile([128, FW], F32, tag="peep")
                        nc.vector.tensor_mul(peep[:, :], c0_sb[j][:, fs],
                                             wc_sb[nm, j][:, :])
                        s = tp.tile([128, FW], F32, tag="s")
                        nc.vector.scalar_tensor_tensor(
                            out=s[:, :], in0=p[:, :], scalar=b_sb[cc][:, :],
                            in1=peep[:, :], op0=ADD, op1=ADD)
                        dst = i_pl if gate == 0 else f_pl
                        nc.scalar.activation(dst[:, fs], s[:, :], SIG)
                    elif gate == 2:                  # g: tanh(p + b)
                        nc.scalar.activation(g_pl[:, fs], p[:, :], TANH,
                                             bias=b_sb[cc][:, :])
                    else:
                        # o-gate pre is in PSUM `p`; finish the cell here.
                        t1 = tp.tile([128, FW], F32, tag="t1")
                        nc.vector.tensor_mul(t1[:, :], i_pl[:, fs], g_pl[:, fs])
                        t2 = tp.tile([128, FW], F32, tag="t2")
                        nc.vector.tensor_mul(t2[:, :], f_pl[:, fs],
                                             c0_sb[j][:, fs])
                        ct_t = op.tile([128, FW], F32, tag="ct_t")
                        nc.vector.tensor_add(ct_t[:, :], t1[:, :], t2[:, :])
                        peep_o = tp.tile([128, FW], F32, tag="peep_o")
                        nc.vector.tensor_mul(peep_o[:, :], ct_t[:, :],
                                             wc_sb["o", j][:, :])
                        so = tp.tile([128, FW], F32, tag="so")
                        nc.vector.scalar_tensor_tensor(
                            out=so[:, :], in0=p[:, :], scalar=b_sb[cc][:, :],
                            in1=peep_o[:, :], op0=ADD, op1=ADD)
                        o_t = op.tile([128, FW], F32, tag="o_t")
                        nc.scalar.activation(o_t[:, :], so[:, :], SIG)
                        th = tp.tile([128, FW], F32, tag="th")
                        nc.scalar.activation(th[:, :], ct_t[:, :], TANH)
                        ht_t = op.tile([128, FW], F32, tag="ht_t")
                        nc.vector.tensor_mul(ht_t[:, :], o_t[:, :], th[:, :])

                        nc.sync.dma_start(out=o_d[js, fs], in_=o_t[:, :])
                        nc.sync.dma_start(out=ht_d[js, fs], in_=ht_t[:, :])
                        nc.sync.dma_start(out=ct_d[js, fs], in_=ct_t[:, :])


_NC_CACHE = None


def _get_nc():
    global _NC_CACHE
    if _NC_CACHE is None:
        _NC_CACHE = _build_nc()
    return _NC_CACHE


def _prep_inputs(input, hidden_state, w_ii, w_if, w_ig, w_io, w_hi, w_hf,
                 w_hg, w_ho, b_i, b_f, b_g, b_o, Wc_i, Wc_f, Wc_o):
    """Host-side reshape/cast. Returns per-core input maps."""
    x = np.ascontiguousarray(np.asarray(input, np.float32))
    hs = np.asarray(hidden_state, np.float32)
    h0 = hs[:, 0]
    c0 = hs[:, 1]

    # reflect-pad then transpose to [ch, n, 10, 10], cast bf16
    xp = np.pad(x, ((0, 0), (0, 0), (1, 1), (1, 1)), mode='reflect')
    hp = np.pad(h0, ((0, 0), (0, 0), (1, 1), (1, 1)), mode='reflect')
    xp = xp.transpose(1, 0, 2, 3).astype(bf16)          # [DIN, N, 10, 10]
    hp = hp.transpose(1, 0, 2, 3).astype(bf16)          # [DH, N, 10, 10]
    c0_t = np.ascontiguousarray(c0.transpose(1, 0, 2, 3)).reshape(DH, N * POS)

    # weights: [4DH, cin, 3, 3] -> [cc, ci, cin128, tap, cout128] -> bf16
    def wprep(ws, cin):
        w = np.concatenate(ws, 0)                       # [2048, cin, 3, 3]
        nci = cin // 128
        w = w.reshape(4 * NJ, 128, nci, 128, 9)          # cc, co, ci, cin, tap
        w = w.transpose(0, 2, 3, 4, 1)                   # cc, ci, cin, tap, co
        return np.ascontiguousarray(w).reshape(4 * NJ, nci, 128, 9 * 128) \
                 .astype(bf16)

    wx = wprep([w_ii, w_if, w_ig, w_io], DIN)
    wh = wprep([w_hi, w_hf, w_hg, w_ho], DH)
    b = np.concatenate([b_i, b_f, b_g, b_o], 0).astype(np.float32) \
          .reshape(4 * DH, 1)
    b = np.ascontiguousarray(b)

    # peephole: [1, DH, 8, 8] -> [DH, 64] -> tile over the 8 n's of a free chunk
    def wcprep(wc):
        w = np.asarray(wc, np.float32).reshape(DH, POS)
        return np.ascontiguousarray(
            np.broadcast_to(w[:, None, :], (DH, FN, POS)).reshape(DH, FW)
        ).astype(bf16)

    wci, wcf, wco = wcprep(Wc_i), wcprep(Wc_f), wcprep(Wc_o)

    in_maps = []
    for k in range(N_CORES):
        ns = slice(k * NB, (k + 1) * NB)
        in_maps.append({
            "xp": np.ascontiguousarray(xp[:, ns]).reshape(DIN, NB * PAD),
            "hp": np.ascontiguousarray(hp[:, ns]).reshape(DH, NB * PAD),
            "c0": np.ascontiguousarray(
                c0_t.reshape(DH, N_CORES, FREE)[:, k, :]),
            "wx": wx, "wh": wh, "wci": wci, "wcf": wcf, "wco": wco, "b": b,
        })
    return in_maps


def _assemble(results):
    """Per-core [DH, NB*64] outputs -> full [N, ...] arrays."""
    def gather(name):
        parts = [results[k][name].reshape(DH, NB, W, W) for k in range(N_CORES)]
        return np.concatenate(parts, axis=1).transpose(1, 0, 2, 3)
    o = gather("o").astype(np.float32)
    ht = gather("ht").astype(np.float32)
    ct = gather("ct").astype(np.float32)
    return o, np.ascontiguousarray(np.stack([ht, ct], axis=1))


def kernel(**inputs):
    nc = _get_nc()
    in_maps = _prep_inputs(**inputs)
    res = run_bass_kernel_spmd(nc, in_maps, list(range(N_CORES)))
    return _assemble(res.results)


if __name__ == "__main__":
    import reference
    inputs = {k: np.asarray(v) for k, v in reference.setup_inputs().items()}
    o, hs = kernel(**inputs)
    print("o", o.shape, "hs", hs.shape)
